# revision 13
# baseline (speedup 1.0000x reference)
"""Trainium2 Bass kernel for nn_Mlp_70798240907434 (content-gated conv MLP).

Sharding: 8 cores = 4 batches x 2 spatial halves (rows 0-47 / 48-95).
Each core computes full layer-1 z for its batch (the global max-pool feeding
the dynamic-kernel generation needs it), gelu-evicts only its own half (+1
halo row) of h, then computes its half of the 3x3 dynamic conv (layer 2).
One SPMD program; the half enters via an If/Else on partition parity.

Key design points:
- bf16 data path; x host-cast + packed [128, 4608] (channel x half on
  partitions).
- 2 x 32x32 max-pool (gl2) is taken on PRE-gelu z straight from PSUM and
  gelu is applied to the 9 pooled values afterwards. Valid because gelu is
  increasing on [-0.75, inf) and <= 0 for z <= 0, so blockmax(gelu(z)) ==
  gelu(blockmax(z)) whenever blockmax(z) >= 0 (verified: min blockmax z =
  0.159 for this problem's inputs, >> bf16 noise).
- pool split across DVE (tensor_reduce) and GpSimd (tensor_tensor max tree).
- all sigmoids via 0.5*(1+tanh(0.5*x)): gelu+tanh share one act table.
- PE warm-up matmuls during the input DMA to hold the tensor-engine pstate.

Self-contained: hardcodes shapes from the problem spec.
"""

import contextlib

import ml_dtypes
import numpy as np

import concourse.bass as bass
import concourse.mybir as mybir
import concourse.tile as tile
from concourse import bacc
from concourse.bass_utils import run_bass_kernel_spmd

F32 = mybir.dt.float32
BF16 = mybir.dt.bfloat16
AF = mybir.ActivationFunctionType

B, CIN, CHID, COUT, H, W = 4, 64, 256, 64, 96, 96
S = H * W                      # 9216
HALF_ROWS = H // 2             # 48
HALF = HALF_ROWS * W           # 4608

PW = W + 2                     # 98
HB = 1
HPF = HB + PW * PW + 3         # 9608

NXCH = 6
XCH = 768                      # 8 rows per half per chunk

L2_ROWS = 5
L2_TILES = [(t0, min(L2_ROWS, HALF_ROWS - t0)) for t0 in range(0, HALF_ROWS, L2_ROWS)]

# const blob column offsets (bf16 [128, CBLOB])
O_W1TH = 0
O_BD1 = 256
O_IDENT = 512
O_W2TH0 = 640
O_W2TH1 = 1216
O_BD2_0 = 1792
O_BD2_1 = 1856
O_CEWT = 1920
O_GDTH = 1925
O_GD2XH = 1934
O_ON5 = 2510
O_ON64 = 2511
O_ON128 = 2575
CBLOB = 2704

N_WARM = 24


def _build():
    nc = bacc.Bacc()

    x2 = nc.declare_dram_parameter("x2", [128, HALF], BF16, isOutput=False)
    blob = nc.declare_dram_parameter("blob", [128, CBLOB], BF16, isOutput=False)
    blob32 = nc.declare_dram_parameter("blob32", [CIN, 4], F32, isOutput=False)
    y = nc.declare_dram_parameter("y", [COUT, HALF], F32, isOutput=True)

    with tile.TileContext(nc) as tc, contextlib.ExitStack() as ctx:
        consts = ctx.enter_context(tc.tile_pool(name="consts", bufs=1))
        big = ctx.enter_context(tc.tile_pool(name="big", bufs=1))
        small = ctx.enter_context(tc.tile_pool(name="small", bufs=2))

        # ---- x DMAs first (sync + gpsimd queues) ----
        xch = [consts.tile([128, XCH], BF16, tag=f"xch{k}", name=f"xch{k}")
               for k in range(NXCH)]
        blob_sb = consts.tile([128, CBLOB], BF16, tag="blob")
        b32_sb = consts.tile([CIN, 4], F32, tag="b32")
        bb = blob_sb[:]
        # x on sync/gpsimd/scalar queues; early blob parts slot in between
        xq = [nc.sync, nc.gpsimd, nc.scalar]
        for k in range(NXCH):
            xq[k % 3].dma_start(xch[k][:], x2[:, k * XCH:(k + 1) * XCH])
            if k == 2:
                nc.scalar.dma_start(bb[:, 0:O_W2TH0], blob[:, 0:O_W2TH0])
                nc.scalar.dma_start(b32_sb[:], blob32[:])
        nc.scalar.dma_start(bb[:, O_ON5:CBLOB], blob[:, O_ON5:CBLOB])
        nc.scalar.dma_start(bb[:, O_W2TH0:O_ON5], blob[:, O_W2TH0:O_ON5])

        w1th = bb[0:64, O_W1TH:O_W1TH + 256]
        bd1 = bb[0:64, O_BD1:O_BD1 + 256]
        ident = bb[0:128, O_IDENT:O_IDENT + 128]
        w2th = [bb[0:128, O_W2TH0:O_W2TH0 + 576], bb[0:128, O_W2TH1:O_W2TH1 + 576]]
        bd2 = [bb[0:128, O_BD2_0:O_BD2_0 + 64], bb[0:128, O_BD2_1:O_BD2_1 + 64]]
        cewt = bb[0:9, O_CEWT:O_CEWT + 5]
        gdth = bb[0:5, O_GDTH:O_GDTH + 9]
        gd2xh = bb[0:5, O_GD2XH:O_GD2XH + 576]
        on5 = bb[0:5, O_ON5:O_ON5 + 1]
        one11 = bb[0:1, O_ON5:O_ON5 + 1]
        on64 = bb[0:1, O_ON64:O_ON64 + 64]
        on128 = bb[0:1, O_ON128:O_ON128 + 128]
        ce1v = b32_sb[:][0:1, 0:1]
        gd1h = b32_sb[:][0:64, 1:2]
        gd21h = b32_sb[:][0:64, 2:3]

        # ---- warm tile, act-table pin, PE warm-up ----
        warm = consts.tile([128, 512], BF16, tag="warm")
        nc.vector.memset(warm[:], 0.0)
        acttab = small.tile([1, 1], F32, tag="acttab")
        nc.scalar.activation(acttab[:], warm[:][0:1, 0:1], AF.Gelu)
        ps_warm = tc.alloc_tile_pool(name="ps_warm", bufs=1, space="PSUM")
        wps = ps_warm.tile([128, 512], F32, tag="w")
        for _ in range(N_WARM):
            nc.tensor.matmul(wps[:], warm[:, 0:128], warm[:],
                             start=True, stop=True)

        # ---- hpad pad zeroing ----
        hpad = [big.tile([128, HPF], BF16, tag=f"hpad{m}", name=f"hpad{m}")
                for m in range(2)]
        for m in range(2):
            hp = hpad[m][:]
            nc.vector.memset(hp[:, 0:HB + PW], 0.0)
            nc.vector.memset(hp[:, HB + 97 * PW:HPF], 0.0)
            colpad = bass.AP(
                tensor=hp.tensor, offset=HB + PW,
                ap=[list(hp.ap[0]), [PW, 96], [97, 2]])
            nc.vector.memset(colpad, 0.0)

        # ---- gl1: global per-channel max of x (bf16 exact for bf16 x) ----
        xmx = small.tile([128, 3 * XCH], BF16, tag="xmx")
        for p in range(3):
            nc.vector.tensor_tensor(xmx[:, p * XCH:(p + 1) * XCH],
                                    xch[2 * p][:], xch[2 * p + 1][:],
                                    op=mybir.AluOpType.max)
        nc.vector.tensor_tensor(xmx[:, 0:XCH], xmx[:, 0:XCH],
                                xmx[:, XCH:2 * XCH], op=mybir.AluOpType.max)
        nc.vector.tensor_tensor(xmx[:, 0:XCH], xmx[:, 0:XCH],
                                xmx[:, 2 * XCH:3 * XCH], op=mybir.AluOpType.max)
        gl128 = small.tile([128, 1], BF16, tag="gl128")
        nc.vector.reduce_max(gl128[:], xmx[:, 0:XCH],
                             axis=mybir.AxisListType.X)
        # cross-half combine via PE transpose (partition dim -> free dim)
        ps_a = tc.alloc_tile_pool(name="ps_a", bufs=1, space="PSUM")
        tp1 = ps_a.tile([1, 128], BF16, tag="a1", name="tp1", bufs=1)
        nc.tensor.transpose(tp1[:], gl128[:], ident)
        glrow = small.tile([1, 128], BF16, tag="glrow")
        nc.vector.tensor_copy(glrow[:], tp1[:])
        glr = small.tile([1, CIN], BF16, tag="glr")
        nc.vector.tensor_tensor(glr[:], glrow[:][0:1, 0:64],
                                glrow[:][0:1, 64:128], op=mybir.AluOpType.max)
        rce1r = small.tile([1, CIN], BF16, tag="rce1r")
        nc.vector.tensor_scalar(rce1r[:], glr[:], ce1v, 0.0,
                                mybir.AluOpType.mult, mybir.AluOpType.max)
        rce1_ps = ps_a.tile([CIN, 1], F32, tag="a2", name="rce1_ps", bufs=1)
        nc.tensor.matmul(rce1_ps[:], rce1r[:], one11, start=True, stop=True)
        rce1 = small.tile([CIN, 1], BF16, tag="rce1")
        nc.vector.tensor_copy(rce1[:], rce1_ps[:])
        outc = small.tile([CIN, 1], F32, tag="outc")
        nc.vector.tensor_scalar_mul(outc[:], rce1_ps[:], gd1h)

        # ---- dyn1 (sigmoid == 0.5 + 0.5*tanh(0.5*arg)) ----
        ocp0_ps = ps_a.tile([1, CHID], F32, tag="a3", name="ocp0_ps", bufs=1)
        nc.tensor.matmul(ocp0_ps[:], rce1[:], bd1, start=True, stop=True)
        rocp1 = small.tile([1, CHID], BF16, tag="rocp1")
        nc.vector.tensor_scalar_max(rocp1[:], ocp0_ps[:], 0.0)
        sig1_ps = ps_a.tile([CIN, CHID], F32, tag="a4", name="sig1_ps", bufs=1)
        nc.tensor.matmul(sig1_ps[:], on64, rocp1[:], start=True, stop=True)
        t1 = small.tile([CIN, CHID], BF16, tag="t1")
        nc.scalar.activation(t1[:], sig1_ps[:], AF.Tanh,
                             bias=outc[:], scale=gd21h)
        dyn1 = consts.tile([128, CHID], BF16, tag="dyn1")
        nc.vector.scalar_tensor_tensor(dyn1[:][0:64, :], t1[:], 1.0, w1th,
                                       mybir.AluOpType.add,
                                       mybir.AluOpType.mult)
        nc.gpsimd.dma_start(dyn1[:][64:128, :], dyn1[:][0:64, :])

        # ---- main phase: everything below depends on the core's half ----
        ps_a.release()
        ps_warm.release()
        ps_big = tc.alloc_tile_pool(name="ps_big", bufs=2, space="PSUM")
        stageA = [big.tile([128, 18], BF16, tag=f"stA{m}", name=f"stA{m}")
                  for m in range(2)]
        gpscr = [small.tile([128, 2880], BF16, tag="gpscr", name=f"gpscr{i}")
                 for i in range(2)]

        pid = nc.partition_id()
        halfsel = nc.snap(pid % 2, min_val=0, max_val=1)

        # ---- layer 1: z = dyn1.T @ x ; gelu -> hpad ; 32x32 max pool ----
        # Pool stage A runs on DVE as a max tree over PAIRS of 16-row
        # supertiles: tensor_tensor bf16 gets the 2x DVE mode (tensor_reduce
        # does not), so 4 TTs (1536+768+384+192) + one 192-elem reduce beat
        # two 1536-elem reduces by ~35%.
        for m in range(2):
            for hb in range(2):
                for j in range(3):
                    z = ps_big.tile([128, 1536], F32, tag="z",
                                    name=f"z{j}_{hb}_{m}")
                    lhs = dyn1[:][64 * hb:64 * hb + 64, 128 * m:128 * m + 128]
                    for i in range(6):
                        ck = 2 * j + i // 3
                        c0 = (i % 3) * 256
                        nc.tensor.matmul(
                            z[:, 256 * i:256 * (i + 1)], lhs,
                            xch[ck][:][64 * hb:64 * hb + 64, c0:c0 + 256],
                            start=True, stop=True)
                    zap = z[:]
                    row0 = 48 * hb + 16 * j
                    hp = hpad[m][:]
                    hoff = HB + (row0 + 1) * PW + 1
                    dst = bass.AP(tensor=hp.tensor, offset=hoff,
                                  ap=[list(hp.ap[0]), [PW, 16], [1, W]])
                    src = bass.AP(tensor=zap.tensor, offset=zap.offset,
                                  ap=[list(zap.ap[0]), [W, 16], [1, W]])
                    nc.scalar.activation(dst, src, AF.Gelu)
                    t = hb * 3 + j
                    if t % 2 == 1:
                        # pair (t-1, t) complete for this m: max tree
                        r0 = 16 * (t - 1)
                        poff = HB + (r0 + 1) * PW + 1
                        g = gpscr[t // 2 % 2][:]
                        nc.vector.tensor_tensor(
                            g[:, 0:1536],
                            bass.AP(tensor=hp.tensor, offset=poff,
                                    ap=[list(hp.ap[0]), [16 * PW, 2],
                                        [PW, 8], [1, W]]),
                            bass.AP(tensor=hp.tensor, offset=poff + 8 * PW,
                                    ap=[list(hp.ap[0]), [16 * PW, 2],
                                        [PW, 8], [1, W]]),
                            op=mybir.AluOpType.max)
                        nc.vector.tensor_tensor(
                            g[:, 1536:2304],
                            bass.AP(tensor=g.tensor, offset=g.offset,
                                    ap=[list(g.ap[0]), [768, 2], [96, 4], [1, W]]),
                            bass.AP(tensor=g.tensor, offset=g.offset + 384,
                                    ap=[list(g.ap[0]), [768, 2], [96, 4], [1, W]]),
                            op=mybir.AluOpType.max)
                        nc.vector.tensor_tensor(
                            g[:, 2304:2688],
                            bass.AP(tensor=g.tensor, offset=g.offset + 1536,
                                    ap=[list(g.ap[0]), [384, 2], [96, 2], [1, W]]),
                            bass.AP(tensor=g.tensor, offset=g.offset + 1536 + 192,
                                    ap=[list(g.ap[0]), [384, 2], [96, 2], [1, W]]),
                            op=mybir.AluOpType.max)
                        nc.vector.tensor_tensor(
                            g[:, 2688:2880],
                            bass.AP(tensor=g.tensor, offset=g.offset + 2304,
                                    ap=[list(g.ap[0]), [192, 2], [1, W]]),
                            bass.AP(tensor=g.tensor, offset=g.offset + 2304 + 96,
                                    ap=[list(g.ap[0]), [192, 2], [1, W]]),
                            op=mybir.AluOpType.max)
                        fin = bass.AP(tensor=g.tensor, offset=g.offset + 2688,
                                      ap=[list(g.ap[0]), [96, 2], [32, 3], [1, 32]])
                        nc.vector.reduce_max(
                            stageA[m][:, 3 * (t - 1):3 * (t - 1) + 6], fin,
                            axis=mybir.AxisListType.X)
        ps_big.release()

        # ---- pool stage B -> gl2 [128, 9] per ctile ----
        gl2 = [small.tile([128, 9], BF16, tag=f"gl2_{m}", name=f"gl2_{m}")
               for m in range(2)]
        for m in range(2):
            sA = stageA[m][:]
            pin = bass.AP(tensor=sA.tensor, offset=sA.offset,
                          ap=[list(sA.ap[0]), [6, 3], [1, 3], [3, 2]])
            nc.vector.reduce_max(gl2[m][:], pin, axis=mybir.AxisListType.X)

        # ---- dyn2 generation (half-agnostic) ----
        ps_c = tc.alloc_tile_pool(name="ps_c", bufs=2, space="PSUM")
        gl2t = small.tile([9, CHID], BF16, tag="gl2t")
        for m in range(2):
            tp_ps = ps_c.tile([9, 128], BF16, tag="c", name="tp_ps", bufs=1)
            nc.tensor.transpose(tp_ps[:], gl2[m][:], ident)
            nc.vector.tensor_copy(gl2t[:, m * 128:(m + 1) * 128], tp_ps[:])
        wu_ps = ps_c.tile([128, 256], F32, tag="wu", name="wu_ps", bufs=1)

        def wu(n):
            for _ in range(n):
                nc.tensor.matmul(wu_ps[:], gl2t[:, 0:128], gl2t[:, 0:256],
                                 start=True, stop=True)

        ce2t_ps = ps_c.tile([5, CHID], F32, tag="c2", name="ce2t_ps", bufs=1)
        rce2t = small.tile([5, CHID], BF16, tag="rce2t")
        for m in range(2):
            nc.tensor.matmul(ce2t_ps[:, m * 128:(m + 1) * 128], cewt,
                             gl2t[:, m * 128:(m + 1) * 128],
                             start=True, stop=True)
            nc.vector.tensor_scalar_max(rce2t[:, m * 128:(m + 1) * 128],
                                        ce2t_ps[:, m * 128:(m + 1) * 128], 0.0)
        wu(2)
        ocp0t_ps = ps_c.tile([5, COUT], F32, tag="c3", name="ocp0t_ps", bufs=1)
        rce2c = [small.tile([128, 5], BF16, tag=f"rce2c{m}", name=f"rce2c{m}")
                 for m in range(2)]
        for m in range(2):
            c_ps = ps_c.tile([128, 5], F32, tag="c", name="c_ps", bufs=1)
            nc.tensor.matmul(c_ps[:], gl2t[:, m * 128:(m + 1) * 128], cewt,
                             start=True, stop=True)
            nc.vector.tensor_scalar_max(rce2c[m][:], c_ps[:], 0.0)
        wu(2)
        for m in range(2):
            nc.tensor.matmul(ocp0t_ps[:], rce2c[m][:], bd2[m],
                             start=(m == 0), stop=(m == 1))
        wu(2)
        rocp2 = small.tile([5, COUT], BF16, tag="rocp2")
        nc.vector.tensor_scalar_max(rocp2[:], ocp0t_ps[:], 0.0)
        gr = small.tile([5, 9 * COUT], BF16, tag="gr")
        rocp_b = bass.AP(tensor=rocp2[:].tensor, offset=rocp2[:].offset,
                         ap=[list(rocp2[:].ap[0]), [0, 9], [1, COUT]])
        nc.vector.tensor_mul(gr[:], rocp_b, gd2xh)
        ocprow_ps = ps_c.tile([1, 9 * COUT], F32, tag="c2",
                              name="ocprow_ps", bufs=1)
        nc.tensor.matmul(ocprow_ps[:, 0:512], on5, gr[:, 0:512],
                         start=True, stop=True)
        nc.tensor.matmul(ocprow_ps[:, 512:576], on5, gr[:, 512:576],
                         start=True, stop=True)
        ocprow = small.tile([1, 9 * COUT], BF16, tag="ocprow")
        nc.vector.tensor_copy(ocprow[:], ocprow_ps[:])
        wu(2)
        outtt = [small.tile([128, 9], BF16, tag=f"outtt{m}", name=f"outtt{m}")
                 for m in range(2)]
        for m in range(2):
            o_ps = ps_c.tile([128, 9], F32, tag="c", name="o_ps", bufs=1)
            nc.tensor.matmul(o_ps[:], rce2t[:, m * 128:(m + 1) * 128], gdth,
                             start=True, stop=True)
            nc.vector.tensor_copy(outtt[m][:], o_ps[:])
        dyn2 = [small.tile([128, 9 * COUT], BF16, tag=f"dyn2_{m}",
                           name=f"dyn2_{m}") for m in range(2)]
        for m in range(2):
            bc_ps = ps_c.tile([128, 9 * COUT], F32, tag="c4",
                              name="bc_ps", bufs=1)
            nc.tensor.matmul(bc_ps[:, 0:512], on128, ocprow[:, 0:512],
                             start=True, stop=True)
            nc.tensor.matmul(bc_ps[:, 512:576], on128, ocprow[:, 512:576],
                             start=True, stop=True)
            s_sb = small.tile([128, 9 * COUT], BF16, tag="s_sb")
            ott = outtt[m][:]
            ott_b = bass.AP(tensor=ott.tensor, offset=ott.offset,
                            ap=[list(ott.ap[0]), [1, 9], [0, COUT]])
            nc.vector.tensor_add(s_sb[:], bc_ps[:], ott_b)
            wu(4)
            t2 = small.tile([128, 9 * COUT], BF16, tag="t2")
            nc.scalar.activation(t2[:], s_sb[:], AF.Tanh)
            nc.vector.scalar_tensor_tensor(dyn2[m][:], t2[:], 1.0, w2th[m],
                                           mybir.AluOpType.add,
                                           mybir.AluOpType.mult)
        ps_c.release()

        # ---- layer 2: 3x3 dynamic conv over own half ----
        ps_y = tc.alloc_tile_pool(name="ps_y", bufs=6, space="PSUM")

        def l2_phase(own):
            r0 = own * HALF_ROWS
            for t0, R in L2_TILES:
                n = PW * R
                yp = ps_y.tile([COUT, n], F32, tag="yp", name=f"yp{own}_{t0}")
                k = 0
                for m in range(2):
                    for di in range(3):
                        for dj in range(3):
                            base = HB + (r0 + t0 + di) * PW + dj - 1
                            nc.tensor.matmul(
                                yp[:],
                                dyn2[m][:, (3 * di + dj) * COUT:
                                        (3 * di + dj + 1) * COUT],
                                hpad[m][:, base:base + n],
                                start=(k == 0), stop=(k == 17))
                            k += 1
                ysb = small.tile([COUT, R * W], F32, tag="ysb",
                                 name=f"ysb{own}_{t0}")
                s2 = bass.AP(tensor=yp[:].tensor, offset=yp[:].offset + 1,
                             ap=[list(yp[:].ap[0]), [PW, R], [1, W]])
                nc.scalar.activation(ysb[:], s2, AF.Copy)
                nc.sync.dma_start(y[:, t0 * W:(t0 + R) * W], ysb[:])

        with tc.If(halfsel < 1) as cmp2:
            l2_phase(0)
        with cmp2.Else():
            l2_phase(1)
        ps_y.release()

    nc.finalize()
    return nc


_CACHE = {}


def _get_nc():
    if "nc" not in _CACHE:
        _CACHE["nc"] = _build()
    return _CACHE["nc"]


def _host_weights(fc1_weight, fc1_ce, fc1_gd, fc1_gd2, fc1_ci,
                  fc2_weight, fc2_ce, fc2_gd, fc2_gd2, fc2_ci):
    f = np.float32
    blob = np.zeros((128, CBLOB), f)
    w1t = fc1_weight.reshape(CHID, CIN).T.astype(f)
    blob[0:64, O_W1TH:O_W1TH + 256] = 0.5 * w1t
    bd1 = np.zeros((CIN, CHID), f)
    for c in range(CIN):
        p, g = c // 8, c % 8
        bd1[c, p * 32:(p + 1) * 32] = fc1_ci[:, g]
    blob[0:64, O_BD1:O_BD1 + 256] = bd1
    blob[0:128, O_IDENT:O_IDENT + 128] = np.eye(128, dtype=f)
    w2t = np.ascontiguousarray(
        fc2_weight.reshape(COUT, CHID, 9).transpose(1, 2, 0).reshape(CHID, 9 * COUT)
    ).astype(f)
    blob[0:128, O_W2TH0:O_W2TH0 + 576] = 0.5 * w2t[0:128]
    blob[0:128, O_W2TH1:O_W2TH1 + 576] = 0.5 * w2t[128:256]
    bd2 = np.zeros((CHID, COUT), f)
    for c in range(CHID):
        p, g = c // 8, c % 8
        bd2[c, p * 2:p * 2 + 2] = fc2_ci[:, g]
    blob[0:128, O_BD2_0:O_BD2_0 + 64] = bd2[0:128]
    blob[0:128, O_BD2_1:O_BD2_1 + 64] = bd2[128:256]
    blob[0:9, O_CEWT:O_CEWT + 5] = fc2_ce.T.astype(f)
    blob[0:5, O_GDTH:O_GDTH + 9] = 0.5 * fc2_gd.T.astype(f)
    blob[0:5, O_GD2XH:O_GD2XH + 576] = 0.5 * np.repeat(
        fc2_gd2.T.astype(f), COUT, axis=1)
    blob[0:5, O_ON5] = 1.0
    blob[0, O_ON64:O_ON64 + 64] = 1.0
    blob[0, O_ON128:O_ON128 + 128] = 1.0
    b32 = np.zeros((CIN, 4), f)
    b32[:, 0] = fc1_ce[0, 0]
    b32[:, 1] = 0.5 * fc1_gd[0, 0]
    b32[:, 2] = 0.5 * fc1_gd2[0, 0]
    return {
        "blob": blob.astype(ml_dtypes.bfloat16),
        "blob32": b32,
    }


def run(inputs, trace=False):
    nc = _get_nc()
    shared = _host_weights(
        inputs["fc1_weight"], inputs["fc1_ce"], inputs["fc1_gd"],
        inputs["fc1_gd2"], inputs["fc1_ci"], inputs["fc2_weight"],
        inputs["fc2_ce"], inputs["fc2_gd"], inputs["fc2_gd2"], inputs["fc2_ci"])
    x = np.asarray(inputs["x"], np.float32)
    in_maps = []
    xb_cache = {}
    for core in range(8):
        bi = core // 2
        if bi not in xb_cache:
            xb = x[bi]
            xb_cache[bi] = np.concatenate(
                [xb[:, :HALF_ROWS, :].reshape(CIN, HALF),
                 xb[:, HALF_ROWS:, :].reshape(CIN, HALF)],
                axis=0).astype(ml_dtypes.bfloat16)
        in_maps.append({"x2": xb_cache[bi], **shared})
    res = run_bass_kernel_spmd(nc, in_maps, list(range(8)), trace=trace)
    out = np.empty((B, COUT, H, W), np.float32)
    for core in range(8):
        bi, half = core // 2, core % 2
        out[bi, :, half * HALF_ROWS:(half + 1) * HALF_ROWS, :] = (
            res.results[core]["y"].reshape(COUT, HALF_ROWS, W))
    return out, res


def kernel(**inputs):
    out, _ = run(inputs, trace=False)
    return out
